# revision 37
# baseline (speedup 1.0000x reference)
"""Trainium2 Bass kernel for a small decoder block (nn_Decoder_75849122448079).

Math (N=4096 seq, W=512 width, P=64 proj, H=8 heads, F=2048 ffn):
  masked_mh = softmax(q_m k_m^T / 8) v_m @ w_o_sum      (w_o_sum = sum of H row-blocks of w_o)
  mh        = softmax(q_c k_c^T / 8) v_c @ w_o_sum      (q_c from masked_mh; k_c/v_c from x)
  h   = LN(mh + x) * g + b
  y   = LeakyReLU(h @ w1 + b1) @ w2 + b2
  out = LN(y + h) * g + b

Sharding: data-parallel over sequence rows — each of the 8 cores owns 512 query
rows end-to-end; K/V projections for the full sequence are replicated on every
core (no collectives). The host only slices x and re-lays-out / dtype-casts
weights (pure marshalling).

Fast path (requires ln_b == ffn_b1 == ffn_b2 == 0, which setup_inputs
guarantees; ln_g is handled generally): the middle LayerNorm's 1/std factor r
cancels exactly —
    y + h = r * (z2 + vc*g),  z2 = LeakyReLU(vc @ (g*w1)) @ w2,  vc = v - mean(v)
because LeakyReLU is positively homogeneous and LN is scale-invariant per row.
So no h, no middle LN, and no [q,w]->[w,q] transposes of h are ever computed.
Mean-centering rides as a rank-1 augmented row through the wosum matmuls, and
masked_mh is never materialized: q_c = A_mn @ (v_m @ wosum @ w_q_c) via the
precomputed 64x64 fold woq.
"""

import os

import numpy as np

import concourse.bass as bass
import concourse.bacc as bacc
import concourse.mybir as mybir
import concourse.tile as tile
from concourse.bass_utils import run_bass_kernel_spmd
from concourse.masks import make_identity

N, W, P, H, F = 4096, 512, 64, 8, 2048
# kt owning PE rows 0:64 / 64:128 of score group g (see K^T packing)
KT_TOP = [kt for sg in range(0, 32 // 4, 2) for kt in range(4 * sg, 4 * sg + 4)]
KT_BOT = [kt for sg in range(1, 32 // 4, 2) for kt in range(4 * sg, 4 * sg + 4)]
NCORES = 8
R = N // NCORES          # 512 rows per core
RT = R // 128            # 4 row tiles per core
WC = W // 128            # 4 contraction chunks over width
ST = N // 128            # 32 sequence (key) tiles
FC = F // 128            # 16 ffn-hidden tiles
NSG = 8                  # x^T DMA chunks (512 tokens each)
EPS = 1e-5
LEAKY = 0.01
SCALE = 0.125            # 1/sqrt(P)

f32 = mybir.dt.float32
bf16 = mybir.dt.bfloat16

MODE = os.environ.get("BASS_DECODER_MODE", "fast")


def _row_bcast(ap, parts=128):
    """AP reading a 1-D DRAM tensor replicated across `parts` partitions."""
    a = ap[:]
    return bass.AP(tensor=a.tensor, offset=a.offset, ap=[[0, parts]] + list(a.ap))


# ======================================================================
# fast path
# ======================================================================

def build_nc_fast():
    cd = bf16
    nc = bacc.Bacc()

    spec = [("x_rows", [128, RT, W], f32),
            ("x_t", [128, NSG, WC, 512], cd),
            ("xr_t", [128, WC, R], cd),
            # packed qkv weights: [w_k2 | w_k2s | w_v2 | w_qm2], each [WC, 2, P]
            ("w_qkv", [128, 4, WC, 2, P], cd),
            ("w_qc", [128, WC, P], cd),
            ("w_oT", [128, WC, W], cd),        # w_o^T, partition-major
            ("g_t", [128, WC], f32),           # ln_g partition-major
            ("ffn_w1", [128, FC, WC, 128], cd),
            ("ffn_w2", [128, FC, W], cd),
            ("ln_g", [W], f32), ("ln_b", [W], f32)]
    t = {}
    for n, s, d in spec:
        t[n] = nc.declare_dram_parameter(n, s, d, isOutput=False)
    t["out"] = nc.declare_dram_parameter("out", [R, W], f32, isOutput=True)

    with tile.TileContext(nc) as tc:
        _build_fast(tc, cd, t)
    return nc


def _build_fast(tc, cd, t):
    nc = tc.nc
    mm = nc.tensor.matmul

    def tp(out, in_, ident):  # PE transpose
        mm(out, in_, ident, is_transpose=True)

    from contextlib import ExitStack
    ctx = ExitStack()
    persist = ctx.enter_context(tc.tile_pool(name="persist", bufs=1))
    stream = ctx.enter_context(tc.tile_pool(name="stream", bufs=2))
    small = ctx.enter_context(tc.tile_pool(name="small", bufs=4))
    pt_pool = ctx.enter_context(tc.tile_pool(name="pt_pool", bufs=3))
    dram = ctx.enter_context(tc.tile_pool(name="dram", bufs=1, space="DRAM"))
    ps_kv = ctx.enter_context(tc.tile_pool(name="ps_kv", bufs=2, space="PSUM"))
    ps_st = ctx.enter_context(tc.tile_pool(name="ps_st", bufs=2, space="PSUM"))
    ps_ac = ctx.enter_context(tc.tile_pool(name="ps_ac", bufs=2, space="PSUM"))

    def big(shape, dtype=f32):        # 1-bank scratch (<=2KB/partition)
        return ps_kv.tile(shape, dtype, tag="kv", name="kvtile")

    def stt_ps(shape, dtype=f32):     # 2-bank score/ffn tiles
        return ps_st.tile(shape, dtype, tag="sT", name="sttile")

    def acc(shape, dtype=f32):        # 1-bank accumulators (aT, y2)
        return ps_ac.tile(shape, dtype, tag="acc", name="acctile")

    # ---------------- critical-path loads (SP queue): qkv weights, x^T ------
    wqkv = persist.tile([128, 4, WC, 2, P], cd)
    nc.sync.dma_start(out=wqkv, in_=t["w_qkv"][:])
    xrT = persist.tile([128, WC, R], cd)
    nc.scalar.dma_start(out=xrT, in_=t["xr_t"][:])
    xT = persist.tile([128, WC, N], cd)
    woT = persist.tile([128, WC, W], cd)

    def xchunk(sg, q):
        q.dma_start(out=xT[:, :, sg * 512:(sg + 1) * 512],
                    in_=t["x_t"][:, sg, :, :])
    xchunk(0, nc.sync)
    xchunk(1, nc.scalar)
    xchunk(2, nc.sync)
    xchunk(3, nc.scalar)
    nc.scalar.dma_start(out=woT, in_=t["w_oT"][:])
    xchunk(4, nc.sync)
    xchunk(5, nc.scalar)
    xchunk(6, nc.sync)
    xchunk(7, nc.scalar)

    # ---------------- small constants / secondary loads on the ACT queue ----
    ident = persist.tile([128, 128], cd)
    make_identity(nc, ident)
    ident_f32 = persist.tile([128, 128], f32)
    make_identity(nc, ident_f32)

    eps_t = persist.tile([128, 1], f32)
    nc.vector.memset(eps_t, EPS)
    zero_t = persist.tile([128, 1], f32)
    nc.vector.memset(zero_t, 0.0)

    wqc = persist.tile([128, WC, P], cd)
    nc.scalar.dma_start(out=wqc, in_=t["w_qc"][:])
    g_rep = persist.tile([128, W], f32)
    nc.scalar.dma_start(out=g_rep, in_=_row_bcast(t["ln_g"]))
    b_rep = persist.tile([128, W], f32)
    nc.scalar.dma_start(out=b_rep, in_=_row_bcast(t["ln_b"]))
    g_t = persist.tile([128, WC], f32)
    nc.scalar.dma_start(out=g_t, in_=t["g_t"][:])

    # K^T / V packed for the attention loops
    G = ST // 2
    kmT = persist.tile([128, G, 128], cd)
    kcT = persist.tile([128, G, 128], cd)
    vm = persist.tile([128, ST, P + 1], cd)
    vc = persist.tile([128, ST, P + 1], cd)
    nc.vector.memset(vm[:, :, P:P + 1], 1.0)
    nc.vector.memset(vc[:, :, P:P + 1], 1.0)

    # PE warm-up: ~3.5us of dummy matmuls while the DMAs land, so the HAM
    # clock gate opens (1.2 -> 2.4 GHz) before the real projections start.
    # f32 runs at 4 cycles/row, so 8 matmuls of 128 columns ~= 3.4us cold.
    ps_warm = big([128, 128])
    for i in range(8):
        mm(ps_warm, ident_f32, ident_f32, start=(i == 0), stop=(i == 7),
           skip_group_check=True)

    # ---------------- Q^T (masked), duplicated into both partition halves ---
    ps_q = big([128, R])
    for wc in range(WC):
        mm(ps_q, wqkv[:, 3, wc, :, :], xrT[:, wc, :], start=(wc == 0), stop=(wc == WC - 1))
    qmT = persist.tile([128, R], cd)
    qm_copy = nc.vector.tensor_copy(qmT, ps_q)

    # ------- replicated K/V projections over the full sequence.  All PSUM
    # evacuation copies go on the Vector engine so the Scalar engine is kept
    # free for the attention exps that overlap this phase.
    last_proj_copy = [None]

    def proj_sgs(sg_lo, sg_hi):
        for sg in range(sg_lo, sg_hi):
            ps_k = big([128, 512])
            wki = 0 if sg % 2 == 0 else 1
            for wc in range(WC):
                mm(ps_k, wqkv[:, wki, wc, :, :], xT[:, wc, sg * 512:(sg + 1) * 512],
                   start=(wc == 0), stop=(wc == WC - 1))
            lo, hi = 4 * (sg // 2), 4 * (sg // 2) + 4
            if sg % 2 == 0:  # top rows = km, bottom = kc
                nc.vector.tensor_copy(kmT[0:64, lo:hi, :], ps_k[0:64, :])
                nc.vector.tensor_copy(kcT[64:128, lo:hi, :], ps_k[64:128, :])
            else:            # top rows = kc, bottom = km
                nc.vector.tensor_copy(kcT[0:64, lo:hi, :], ps_k[0:64, :])
                nc.vector.tensor_copy(kmT[64:128, lo:hi, :], ps_k[64:128, :])
            # all 16 MMs form ONE accumulation group in one PSUM bank: start
            # clears the whole bank once, then each byte is overwritten on its
            # first write and accumulated after (order-independent).
            ps_v4 = big([128, 4, 2, P])
            for st4 in range(4):
                st = 4 * sg + st4
                for wc in range(WC):
                    mm(ps_v4[:, st4, :, :], xT[:, wc, st * 128:(st + 1) * 128],
                       wqkv[:, 2, wc, :, :],
                       start=(st4 == 0 and wc == 0), stop=(st4 == 3 and wc == WC - 1),
                       skip_group_check=True)
            nc.vector.tensor_copy(vm[:, 4 * sg:4 * sg + 4, 0:P], ps_v4[:, :, 0, :])
            last_proj_copy[0] = nc.vector.tensor_copy(
                vc[:, 4 * sg:4 * sg + 4, 0:P], ps_v4[:, :, 1, :])

    proj_sgs(0, 4)

    # ---- w_o-derived constants: emitted between the projection halves so
    # they are ready-compatible (woT lands ~19us) and their DVE ops never
    # block later projection copies or the masked-post chain.
    woTs_f32 = persist.tile([128, WC, P], f32)
    nc.vector.tensor_add(woTs_f32, woT[:, :, 0:P], woT[:, :, P:2 * P])
    for hh in range(2, H):
        nc.vector.tensor_add(woTs_f32, woTs_f32, woT[:, :, hh * P:(hh + 1) * P])
    woTs = persist.tile([128, WC, P], cd)
    nc.vector.tensor_copy(woTs, woTs_f32)

    wosum_aug = persist.tile([P + 1, W], cd)
    nc.vector.memset(wosum_aug[P:P + 1, :], 1.0)
    for wc in range(WC):
        ps_wo = big([P, 128], cd)
        tp(ps_wo, woTs[:, wc, :], ident)
        nc.vector.tensor_copy(wosum_aug[0:P, wc * 128:(wc + 1) * 128], ps_wo)

    # woq = wosum @ w_q_c  [64, 64], then duplicated -> [64, 128]
    ps_woq = big([P, P])
    for wc in range(WC):
        mm(ps_woq, woTs[:, wc, :], wqc[:, wc, :],
           start=(wc == 0), stop=(wc == WC - 1))
    woq_dup = persist.tile([P, 128], cd)
    nc.vector.tensor_copy(woq_dup[:, 0:P], ps_woq)
    nc.vector.tensor_copy(woq_dup[:, P:2 * P], ps_woq)

    proj_sgs(4, 8)

    # ------------------------------------------------------------- attention
    def scores_pair(kT, qT, g):
        sT = stt_ps([128, 2, 512])
        mm(sT[:, 0, :], kT[0:64, g, :], qT[0:64, :], tile_position=(0, 0))
        mm(sT[:, 1, :], kT[64:128, g, :], qT[64:128, :], tile_position=(64, 0))
        return sT

    def attn_run(kT, v, qT, ps_aT, g_lo, g_hi, first, last):
        """Score/exp/accumulate groups [g_lo, g_hi) with one-group lookahead."""
        sT_prev = scores_pair(kT, qT, g_lo)
        for g in range(g_lo + 1, g_hi + 1):
            sT_next = scores_pair(kT, qT, g) if g < g_hi else None
            ptl = pt_pool.tile([128, 2, 512], cd, tag="pt")
            nc.scalar.activation(ptl, sT_prev, mybir.ActivationFunctionType.Exp,
                                 scale=SCALE)
            for j in range(2):
                kt = (KT_TOP, KT_BOT)[j][g - 1]
                mm(ps_aT, v[:, kt, :], ptl[:, j, :],
                   start=(first and g == g_lo + 1 and j == 0),
                   stop=(last and g == g_hi and j == 1))
            sT_prev = sT_next

    # masked attention: all 32 key tiles
    ps_aTm = acc([P + 1, R])
    attn_run(kmT, vm, qmT, ps_aTm, 0, G, True, True)

    amT = persist.tile([P + 1, R], f32, tag="amT", name="amT")
    am_copy = nc.vector.tensor_copy(amT, ps_aTm)

    # ---------------- masked -> cross: normalize A, q_c^T = woq^T @ A_mn^T --
    # normalize mult + evacuations run on the (idle) Scalar engine so the DVE
    # and PE ping-pong stays short.
    amT_n = persist.tile([P, R], cd)
    for qt in range(RT):
        ps_a4 = big([128, P + 1])
        tp(ps_a4, amT[:, qt * 128:(qt + 1) * 128], ident_f32[0:P + 1, 0:P + 1])
        recip_m = small.tile([128, 1], f32, tag="recip")
        nc.vector.reciprocal(recip_m, ps_a4[:, P:P + 1])
        a_m = small.tile([128, P], cd, tag="a_m")
        nc.scalar.activation(a_m, ps_a4[:, 0:P],
                             mybir.ActivationFunctionType.Copy, scale=recip_m)
        ps_at2 = big([P, 128], cd)
        tp(ps_at2, a_m, ident)
        nc.vector.tensor_copy(amT_n[:, qt * 128:(qt + 1) * 128], ps_at2)

    # column-split so each 128-q slice runs as soon as its amT_n slice lands
    qcT = persist.tile([128, R], cd)
    for qt in range(RT):
        ps_qc = big([128, 128])
        mm(ps_qc, woq_dup, amT_n[:, qt * 128:(qt + 1) * 128])
        qc_copy = nc.vector.tensor_copy(qcT[:, qt * 128:(qt + 1) * 128], ps_qc)

    # ---- bulk loads: anchored to the end of the projections, so they use
    # the idle DMA window during masked attention.
    from concourse.bass import _add_dep_helper
    anchor = last_proj_copy[0]
    xr_nat = persist.tile([128, RT, W], f32)
    d0 = nc.scalar.dma_start(out=xr_nat, in_=t["x_rows"][:])
    _add_dep_helper(d0.ins, anchor.ins, sync=True, reason="delay x_rows load")
    w1_all = persist.tile([128, FC, WC, 128], cd)
    d1 = nc.scalar.dma_start(out=w1_all, in_=t["ffn_w1"][:])
    _add_dep_helper(d1.ins, anchor.ins, sync=True, reason="delay ffn w1 preload")
    w2_all = persist.tile([128, FC, W], cd)
    d2 = nc.scalar.dma_start(out=w2_all, in_=t["ffn_w2"][:])
    _add_dep_helper(d2.ins, anchor.ins, sync=True, reason="delay ffn w2 preload")

    # wosumg_aug = wosum_aug * g[w]  (g folded for the residual path)
    wosumg_aug = persist.tile([P + 1, W], cd)
    nc.vector.tensor_mul(wosumg_aug, wosum_aug, g_rep[0:P + 1, :])
    # wsr = sum_w wosum[d, w] -> DRAM roundtrip -> replicated [128, 64]
    wsr = small.tile([P, 1], f32, tag="wsr")
    junk = stream.tile([P, W], cd, tag="junk")
    nc.vector.scalar_tensor_tensor(out=junk, in0=wosum_aug[0:P, :], scalar=1.0,
                                   in1=wosum_aug[0:P, :],
                                   op0=mybir.AluOpType.mult,
                                   op1=mybir.AluOpType.max, accum_out=wsr)
    wsr_dram = dram.tile([P], f32)
    nc.scalar.dma_start(out=wsr_dram, in_=wsr[:, 0])
    wsr_rep = persist.tile([128, P], f32)
    nc.scalar.dma_start(out=wsr_rep, in_=_row_bcast(wsr_dram))

    # ------------------------------------------------------------------ cross
    ps_aTc = acc([P + 1, R])
    attn_run(kcT, vc, qcT, ps_aTc, 0, G, True, True)

    # ---- x-derived prep: runs on the DVE during cross attention.  Each op
    # carries an explicit dependency on the masked-post chain so the static
    # scheduler can never slot it ahead of that critical path.
    xsum = persist.tile([128, RT], f32)
    for qt in range(RT):
        xs_scr = stream.tile([128, W], f32, tag="junk2")
        i0 = nc.vector.scalar_tensor_tensor(out=xs_scr, in0=xr_nat[:, qt, :],
                                            scalar=1.0, in1=xr_nat[:, qt, :],
                                            op0=mybir.AluOpType.mult,
                                            op1=mybir.AluOpType.max,
                                            accum_out=xsum[:, qt:qt + 1])
        _add_dep_helper(i0.ins, qc_copy.ins, sync=True, reason="after masked-post")
    # xg = x * g (residual path), and fold ln_g into ffn_w1 in place
    xg_nat = persist.tile([128, RT, W], f32)
    for qt in range(RT):
        i0 = nc.vector.tensor_mul(xg_nat[:, qt, :], xr_nat[:, qt, :], g_rep)
        _add_dep_helper(i0.ins, qc_copy.ins, sync=True, reason="after masked-post")
    for wc in range(WC):
        i0 = nc.vector.tensor_scalar_mul(w1_all[:, :, wc, :],
                                         w1_all[:, :, wc, :],
                                         g_t[:, wc:wc + 1])
        _add_dep_helper(i0.ins, qc_copy.ins, sync=True, reason="after masked-post")

    acT = persist.tile([P + 1, R], f32, tag="acT", name="acT")
    nc.vector.tensor_copy(acT, ps_aTc)

    # ------- cross post: normalize + augment with the -mean row, transposed
    # acn_aug[0:64, q] = A_cn^T ; acn_aug[64, q] = -mean_w(v[q, :])
    acn_aug = persist.tile([P + 1, R], cd)
    for qt in range(RT):
        ps_a4 = big([128, P + 1])
        tp(ps_a4, acT[:, qt * 128:(qt + 1) * 128], ident_f32[0:P + 1, 0:P + 1])
        recip_c = small.tile([128, 1], f32, tag="recip")
        nc.vector.reciprocal(recip_c, ps_a4[:, P:P + 1])
        a_n = small.tile([128, P + 1], cd, tag="a_n")
        nc.scalar.activation(a_n[:, 0:P], ps_a4[:, 0:P],
                             mybir.ActivationFunctionType.Copy, scale=recip_c)
        # t1[q] = (A_c[q, :] . wsr), normalized inside the aug combine below
        t1 = small.tile([128, 1], f32, tag="t1")
        scr = stream.tile([128, P], f32, tag="mscr")
        nc.vector.scalar_tensor_tensor(out=scr, in0=ps_a4[:, 0:P], scalar=1.0,
                                       in1=wsr_rep, op0=mybir.AluOpType.mult,
                                       op1=mybir.AluOpType.mult, accum_out=t1)
        # aug col = -(t1 * recip + xsum)/W
        t2 = small.tile([128, 1], f32, tag="t2")
        nc.vector.scalar_tensor_tensor(out=t2, in0=t1, scalar=recip_c,
                                       in1=xsum[:, qt:qt + 1],
                                       op0=mybir.AluOpType.mult,
                                       op1=mybir.AluOpType.add)
        nc.vector.tensor_scalar_mul(a_n[:, P:P + 1], t2, -1.0 / W)
        ps_at2 = big([P + 1, 128], cd)
        tp(ps_at2, a_n, ident)
        nc.scalar.copy(acn_aug[:, qt * 128:(qt + 1) * 128], ps_at2)

    # vc^T[w, q] = (wosum_aug^T @ acn_aug) + x^T   (centered v, transposed).
    # The x^T term rides as an identity-matmul accumulate; evacuation goes on
    # the Scalar engine, which is idle between the last exp and first Prelu.
    # Column-halved so work starts before the full acn_aug is assembled.
    vcT = persist.tile([128, WC, R], cd)
    for qh in range(2):
        cols = slice(qh * 256, (qh + 1) * 256)
        for wc in range(WC):
            ps_v1 = stt_ps([128, 256])
            mm(ps_v1, wosum_aug[:, wc * 128:(wc + 1) * 128], acn_aug[:, cols],
               start=True, stop=False, skip_group_check=True)
            mm(ps_v1, ident, xrT[:, wc, cols], start=False, stop=True,
               skip_group_check=True)
            nc.scalar.copy(vcT[:, wc, cols], ps_v1)

    # ------------------------------------------------------------------- FFN
    luT = persist.tile([128, FC, R], cd)
    for fc in range(FC):
        ps_y1 = stt_ps([128, R])
        for wc in range(WC):
            mm(ps_y1, w1_all[:, fc, wc, :], vcT[:, wc, :],
               start=(wc == 0), stop=(wc == WC - 1))
        nc.scalar.activation(luT[:, fc, :], ps_y1,
                             mybir.ActivationFunctionType.Prelu,
                             bias=zero_t, scale=1.0, alpha=LEAKY)

    # vcg[q, w] = (v - mean) * g  (residual for the final LN; not needed until
    # FFN2, so it is emitted after FFN1 to fill PE gaps there)
    vcg = persist.tile([128, RT, W], f32)
    for qt in range(RT):
        ps_vq = big([128, W])
        mm(ps_vq, acn_aug[:, qt * 128:(qt + 1) * 128], wosumg_aug)
        nc.vector.scalar_tensor_tensor(out=vcg[:, qt, :], in0=ps_vq, scalar=1.0,
                                       in1=xg_nat[:, qt, :],
                                       op0=mybir.AluOpType.mult,
                                       op1=mybir.AluOpType.add)

    def ln_finish(dst, v_sb, ssum):
        """dst = LN(v_sb) * g + b, with sum(v) already in ssum [128, 1]."""
        scr = stream.tile([128, W], f32, tag="scr")
        ss2 = small.tile([128, 1], f32, tag="ss2")
        nc.scalar.activation(scr, v_sb, mybir.ActivationFunctionType.Square,
                             accum_out=ss2)
        m = small.tile([128, 1], f32, tag="m")
        nc.vector.tensor_scalar_mul(m, ssum, 1.0 / W)
        var = small.tile([128, 1], f32, tag="var")
        nc.vector.tensor_mul(var, m, m)
        nc.vector.scalar_tensor_tensor(out=var, in0=ss2, scalar=1.0 / W,
                                       in1=var, op0=mybir.AluOpType.mult,
                                       op1=mybir.AluOpType.subtract)
        nc.scalar.activation(var, var, mybir.ActivationFunctionType.Sqrt,
                             bias=eps_t, scale=1.0)
        nc.vector.reciprocal(var, var)
        nc.vector.tensor_scalar(dst, v_sb, scalar1=m, scalar2=var,
                                op0=mybir.AluOpType.subtract,
                                op1=mybir.AluOpType.mult)
        nc.vector.tensor_mul(dst, dst, g_rep)
        nc.vector.tensor_add(dst, dst, b_rep)

    out_re = t["out"].rearrange("(q p) w -> q p w", p=128)
    for qt in range(RT):
        ps_y2 = acc([128, W])
        for fc in range(FC):
            mm(ps_y2, luT[:, fc, qt * 128:(qt + 1) * 128],
               w2_all[:, fc, :], start=(fc == 0), stop=(fc == FC - 1))
        sum2 = stream.tile([128, W], f32, tag="sum")
        ssum = small.tile([128, 1], f32, tag="ssum")
        nc.vector.scalar_tensor_tensor(out=sum2, in0=ps_y2,
                                       scalar=1.0, in1=vcg[:, qt, :],
                                       op0=mybir.AluOpType.mult,
                                       op1=mybir.AluOpType.add,
                                       accum_out=ssum)
        ln_finish(sum2, sum2, ssum)
        nc.sync.dma_start(out=out_re[qt], in_=sum2)

    ctx.close()


def make_in_maps_fast(inputs):
    import ml_dtypes
    wd = ml_dtypes.bfloat16

    def pm(a):  # [(c p), d] -> [p, c, d]  (partition-major for contiguous DMA)
        c = a.shape[0] // 128
        return np.ascontiguousarray(
            a.reshape(c, 128, *a.shape[1:]).transpose(1, 0, 2), dtype=wd)

    f = {k: np.asarray(v, dtype=np.float32) for k, v in inputs.items()}
    x = f["x"]
    x_cd = x.astype(wd)
    xt = x_cd.T  # [W, N]
    w_k2 = np.stack([pm(f["w_k_m"]), pm(f["w_k_c"])], axis=2)
    w_k2s = np.stack([pm(f["w_k_c"]), pm(f["w_k_m"])], axis=2)
    w_v2 = np.stack([pm(f["w_v_m"]), pm(f["w_v_c"])], axis=2)
    w_qm2 = np.stack([pm(f["w_q_m"]), pm(f["w_q_m"])], axis=2)
    shared = {
        # one packed DMA: [w_k2 | w_k2s | w_v2 | w_qm2]
        "w_qkv": np.ascontiguousarray(
            np.stack([w_k2, w_k2s, w_v2, w_qm2], axis=1), dtype=wd),
        "w_qc": pm(f["w_q_c"]),
        # w_o^T [(c p), (h d)] -> [p, c, (h d)]
        "w_oT": np.ascontiguousarray(
            f["w_o"].T.reshape(WC, 128, H * P).transpose(1, 0, 2), dtype=wd),
        "g_t": np.ascontiguousarray(f["ln_g"].reshape(WC, 128).T,
                                    dtype=np.float32),
        # x^T chunk-contiguous: [p, sg, c, j] = x[sg*512+j, c*128+p]
        "x_t": np.ascontiguousarray(
            xt.reshape(WC, 128, NSG, 512).transpose(1, 2, 0, 3), dtype=wd),
        "ffn_w1": np.ascontiguousarray(
            f["ffn_w1"].reshape(WC, 128, FC, 128).transpose(1, 2, 0, 3), dtype=wd),
        "ffn_w2": np.ascontiguousarray(
            f["ffn_w2"].reshape(FC, 128, W).transpose(1, 0, 2), dtype=wd),
        "ln_g": f["ln_g"], "ln_b": f["ln_b"],
    }
    in_maps = []
    for c in range(NCORES):
        m = dict(shared)
        xr = x[c * R:(c + 1) * R]  # [R, W] -> [p, q, w]
        m["x_rows"] = np.ascontiguousarray(
            xr.reshape(RT, 128, W).transpose(1, 0, 2))
        m["xr_t"] = np.ascontiguousarray(
            x_cd.T[:, c * R:(c + 1) * R].reshape(WC, 128, R).transpose(1, 0, 2))
        in_maps.append(m)
    return in_maps


# ======================================================================
# general fallback path (handles nonzero ln_b / ffn_b1 / ffn_b2)
# ======================================================================

def build_nc_general():
    cd = bf16
    nc = bacc.Bacc()

    spec = [("x_rows", [128, RT, W], f32),
            ("x_t", [W, N], cd),
            ("xr_t", [128, WC, R], cd),
            ("w_qm2", [128, WC, 2, P], cd),
            ("w_qc2", [128, WC, 2, P], cd),
            ("w_k2", [128, WC, 2, P], cd),
            ("w_k2s", [128, WC, 2, P], cd),
            ("w_v2", [128, WC, 2, P], cd),
            ("w_o", [64, H, W], cd),
            ("ffn_w1", [128, FC, WC, 128], cd),
            ("ffn_w2", [128, FC, W], cd),
            ("ln_g", [W], f32), ("ln_b", [W], f32),
            ("ffn_b1", [128, FC], f32), ("ffn_b2", [W], f32)]
    t = {}
    for n, s, d in spec:
        t[n] = nc.declare_dram_parameter(n, s, d, isOutput=False)
    t["out"] = nc.declare_dram_parameter("out", [R, W], f32, isOutput=True)

    with tile.TileContext(nc) as tc:
        _build_general(tc, cd, t)
    return nc


def _build_general(tc, cd, t):
    nc = tc.nc
    mm = nc.tensor.matmul

    def tp(out, in_, ident):  # PE transpose
        mm(out, in_, ident, is_transpose=True)

    from contextlib import ExitStack
    ctx = ExitStack()
    persist = ctx.enter_context(tc.tile_pool(name="persist", bufs=1))
    stream = ctx.enter_context(tc.tile_pool(name="stream", bufs=2))
    wstream = ctx.enter_context(tc.tile_pool(name="wstream", bufs=3))
    small = ctx.enter_context(tc.tile_pool(name="small", bufs=4))
    pt_pool = ctx.enter_context(tc.tile_pool(name="pt_pool", bufs=3))
    dram = ctx.enter_context(tc.tile_pool(name="dram", bufs=1, space="DRAM"))
    ps_kv = ctx.enter_context(tc.tile_pool(name="ps_kv", bufs=2, space="PSUM"))
    ps_st = ctx.enter_context(tc.tile_pool(name="ps_st", bufs=2, space="PSUM"))
    ps_ac = ctx.enter_context(tc.tile_pool(name="ps_ac", bufs=2, space="PSUM"))

    def big(shape, dtype=f32):
        return ps_kv.tile(shape, dtype, tag="kv", name="kvtile")

    def stt(shape, dtype=f32):
        return ps_st.tile(shape, dtype, tag="sT", name="sttile")

    def acc(shape, dtype=f32):
        return ps_ac.tile(shape, dtype, tag="acc", name="acctile")

    wqm2 = persist.tile([128, WC, 2, P], cd)
    nc.sync.dma_start(out=wqm2, in_=t["w_qm2"][:])
    wqc2 = persist.tile([128, WC, 2, P], cd)
    nc.sync.dma_start(out=wqc2, in_=t["w_qc2"][:])
    wk2 = persist.tile([128, WC, 2, P], cd)
    nc.sync.dma_start(out=wk2, in_=t["w_k2"][:])
    wk2s = persist.tile([128, WC, 2, P], cd)
    nc.sync.dma_start(out=wk2s, in_=t["w_k2s"][:])
    wv2 = persist.tile([128, WC, 2, P], cd)
    nc.sync.dma_start(out=wv2, in_=t["w_v2"][:])
    xrT = persist.tile([128, WC, R], cd)
    nc.sync.dma_start(out=xrT, in_=t["xr_t"][:])
    xT = persist.tile([128, WC, N], cd)
    x_t_re = t["x_t"].rearrange("(c p) n -> p c n", p=128)
    for sg in range(NSG):
        nc.sync.dma_start(out=xT[:, :, sg * (N // NSG):(sg + 1) * (N // NSG)],
                          in_=x_t_re[:, :, sg * (N // NSG):(sg + 1) * (N // NSG)])

    ident = persist.tile([128, 128], cd)
    make_identity(nc, ident)
    ident_f32 = persist.tile([128, 128], f32)
    make_identity(nc, ident_f32)

    eps_t = persist.tile([128, 1], f32)
    nc.vector.memset(eps_t, EPS)

    g_rep = persist.tile([128, W], f32)
    nc.scalar.dma_start(out=g_rep, in_=_row_bcast(t["ln_g"]))
    b_rep = persist.tile([128, W], f32)
    nc.scalar.dma_start(out=b_rep, in_=_row_bcast(t["ln_b"]))
    b2_rep = persist.tile([128, W], f32)
    nc.scalar.dma_start(out=b2_rep, in_=_row_bcast(t["ffn_b2"]))
    b1_sb = persist.tile([128, FC], f32)
    nc.scalar.dma_start(out=b1_sb, in_=t["ffn_b1"][:])

    wo_stage = stream.tile([64, H, W], cd, tag="wo")
    nc.scalar.dma_start(out=wo_stage, in_=t["w_o"][:])
    wos_f32 = persist.tile([64, W], f32)
    nc.vector.tensor_add(wos_f32, wo_stage[:, 0, :], wo_stage[:, 1, :])
    for hh in range(2, H):
        nc.vector.tensor_add(wos_f32, wos_f32, wo_stage[:, hh, :])
    wosum = persist.tile([64, W], cd)
    nc.vector.tensor_copy(wosum, wos_f32)

    xr_nat = persist.tile([128, RT, W], f32)
    nc.scalar.dma_start(out=xr_nat, in_=t["x_rows"][:])

    G = ST // 2
    kmT = persist.tile([128, G, 128], cd)
    kcT = persist.tile([128, G, 128], cd)
    vm = persist.tile([128, ST, P + 1], cd)
    vc = persist.tile([128, ST, P + 1], cd)
    nc.vector.memset(vm[:, :, P:P + 1], 1.0)
    nc.vector.memset(vc[:, :, P:P + 1], 1.0)

    ps_q = big([128, R])
    for wc in range(WC):
        mm(ps_q, wqm2[:, wc, :, :], xrT[:, wc, :], start=(wc == 0), stop=(wc == WC - 1))
    qmT = persist.tile([128, R], cd)
    qm_copy = nc.vector.tensor_copy(qmT, ps_q)

    def proj_sgs(sg_lo, sg_hi):
        for sg in range(sg_lo, sg_hi):
            ps_k = big([128, 512])
            wk = wk2 if sg % 2 == 0 else wk2s
            for wc in range(WC):
                mm(ps_k, wk[:, wc, :, :], xT[:, wc, sg * 512:(sg + 1) * 512],
                   start=(wc == 0), stop=(wc == WC - 1))
            lo, hi = 4 * (sg // 2), 4 * (sg // 2) + 4
            if sg % 2 == 0:
                nc.scalar.copy(kmT[0:64, lo:hi, :], ps_k[0:64, :])
                nc.vector.tensor_copy(kcT[64:128, lo:hi, :], ps_k[64:128, :])
            else:
                nc.scalar.copy(kcT[0:64, lo:hi, :], ps_k[0:64, :])
                nc.vector.tensor_copy(kmT[64:128, lo:hi, :], ps_k[64:128, :])
            for st in range(4 * sg, 4 * sg + 4):
                ps_v = big([128, 2, P])
                for wc in range(WC):
                    mm(ps_v, xT[:, wc, st * 128:(st + 1) * 128], wv2[:, wc, :, :],
                       start=(wc == 0), stop=(wc == WC - 1))
                nc.scalar.copy(vm[:, st, 0:P], ps_v[:, 0, :])
                nc.vector.tensor_copy(vc[:, st, 0:P], ps_v[:, 1, :])

    from concourse.bass import _add_dep_helper
    w1_all = persist.tile([128, FC, WC, 128], cd)
    d1 = nc.scalar.dma_start(out=w1_all, in_=t["ffn_w1"][:])
    _add_dep_helper(d1.ins, qm_copy.ins, sync=True, reason="delay ffn w1 preload")
    w2_all = persist.tile([128, FC, W], cd)
    d2 = nc.scalar.dma_start(out=w2_all, in_=t["ffn_w2"][:])
    _add_dep_helper(d2.ins, qm_copy.ins, sync=True, reason="delay ffn w2 preload")

    def scores_pair(kT, qT, g):
        sT = stt([128, 2, 512])
        mm(sT[:, 0, :], kT[0:64, g, :], qT[0:64, :])
        mm(sT[:, 1, :], kT[64:128, g, :], qT[64:128, :])
        return sT

    def attn_run(kT, v, qT, ps_aT, g_lo, g_hi, first, last):
        sT_prev = scores_pair(kT, qT, g_lo)
        for g in range(g_lo + 1, g_hi + 1):
            sT_next = scores_pair(kT, qT, g) if g < g_hi else None
            ptl = pt_pool.tile([128, 2, 512], cd, tag="pt")
            nc.scalar.activation(ptl, sT_prev, mybir.ActivationFunctionType.Exp,
                                 scale=SCALE)
            for j in range(2):
                kt = (KT_TOP, KT_BOT)[j][g - 1]
                mm(ps_aT, v[:, kt, :], ptl[:, j, :],
                   start=(first and g == g_lo + 1 and j == 0),
                   stop=(last and g == g_hi and j == 1))
            sT_prev = sT_next

    ps_aTm = acc([P + 1, R])
    proj_sgs(0, 4)
    attn_run(kmT, vm, qmT, ps_aTm, 0, G // 2, True, False)
    proj_sgs(4, 8)
    attn_run(kmT, vm, qmT, ps_aTm, G // 2, G, False, True)
    amT = persist.tile([P + 1, R], f32, tag="amT", name="amT")
    nc.vector.tensor_copy(amT, ps_aTm)

    ps_a4 = big([128, RT, P + 1])
    for qt in range(RT):
        tp(ps_a4[:, qt, :], amT[:, qt * 128:(qt + 1) * 128],
           ident_f32[0:P + 1, 0:P + 1])
    a_m = small.tile([128, RT, P], cd, tag="a_m")
    recip_m = small.tile([128, RT, 1], f32, tag="recip")
    for qt in range(RT):
        nc.vector.reciprocal(recip_m[:, qt, :], ps_a4[:, qt, P:P + 1])
        nc.vector.tensor_scalar_mul(a_m[:, qt, :], ps_a4[:, qt, 0:P],
                                    recip_m[:, qt, :])
    ps_at2 = big([P, R], cd)
    for qt in range(RT):
        tp(ps_at2[:, qt * 128:(qt + 1) * 128], a_m[:, qt, :], ident)
    amT_n = persist.tile([P, R], cd)
    nc.vector.tensor_copy(amT_n, ps_at2)

    mhT = persist.tile([128, WC, R], cd)
    for wc in range(WC):
        ps_mh = stt([128, R])
        mm(ps_mh, wosum[:, wc * 128:(wc + 1) * 128], amT_n)
        nc.vector.tensor_copy(mhT[:, wc, :], ps_mh)

    ps_qc = big([128, R])
    for wc in range(WC):
        mm(ps_qc, wqc2[:, wc, :, :], mhT[:, wc, :], start=(wc == 0), stop=(wc == WC - 1))
    qcT = persist.tile([128, R], cd)
    nc.vector.tensor_copy(qcT, ps_qc)

    ps_aTc = acc([P + 1, R])
    attn_run(kcT, vc, qcT, ps_aTc, 0, G, True, True)
    acT = persist.tile([P + 1, R], f32, tag="acT", name="acT")
    nc.vector.tensor_copy(acT, ps_aTc)

    ps_s1 = big([128, RT, 1])
    for qt in range(RT):
        tp(ps_s1[:, qt, :], acT[P:P + 1, qt * 128:(qt + 1) * 128],
           ident_f32[P:P + 1, P:P + 1])
    rs_c = small.tile([128, RT, 1], f32, tag="rs_c")
    for qt in range(RT):
        nc.vector.reciprocal(rs_c[:, qt, :], ps_s1[:, qt, :])

    acT_cd = persist.tile([P + 1, R], cd)
    nc.vector.tensor_copy(acT_cd, acT)

    h_f32 = persist.tile([128, RT, W], f32)

    def ln_finish(dst, v_sb, ssum):
        scr = stream.tile([128, W], f32, tag="scr")
        ss2 = small.tile([128, 1], f32, tag="ss2")
        nc.scalar.activation(scr, v_sb, mybir.ActivationFunctionType.Square,
                             accum_out=ss2)
        m = small.tile([128, 1], f32, tag="m")
        nc.vector.tensor_scalar_mul(m, ssum, 1.0 / W)
        var = small.tile([128, 1], f32, tag="var")
        nc.vector.tensor_mul(var, m, m)
        nc.vector.scalar_tensor_tensor(out=var, in0=ss2, scalar=1.0 / W,
                                       in1=var, op0=mybir.AluOpType.mult,
                                       op1=mybir.AluOpType.subtract)
        nc.scalar.activation(var, var, mybir.ActivationFunctionType.Sqrt,
                             bias=eps_t, scale=1.0)
        nc.vector.reciprocal(var, var)
        nc.vector.tensor_scalar(dst, v_sb, scalar1=m, scalar2=var,
                                op0=mybir.AluOpType.subtract,
                                op1=mybir.AluOpType.mult)
        nc.vector.tensor_mul(dst, dst, g_rep)
        nc.vector.tensor_add(dst, dst, b_rep)

    for qt in range(RT):
        ps_mhc = stt([128, W])
        mm(ps_mhc, acT_cd[0:P, qt * 128:(qt + 1) * 128], wosum)
        sum_sb = stream.tile([128, W], f32, tag="sum")
        ssum = small.tile([128, 1], f32, tag="ssum")
        nc.vector.scalar_tensor_tensor(out=sum_sb, in0=ps_mhc,
                                       scalar=rs_c[:, qt, :],
                                       in1=xr_nat[:, qt, :],
                                       op0=mybir.AluOpType.mult,
                                       op1=mybir.AluOpType.add,
                                       accum_out=ssum)
        ln_finish(h_f32[:, qt, :], sum_sb, ssum)

    h_cd = persist.tile([128, RT, W], cd)
    nc.vector.tensor_copy(h_cd, h_f32)

    hT = persist.tile([128, WC, R], cd)
    for qt in range(RT):
        pst = big([128, WC, 128], cd)
        for wc in range(WC):
            tp(pst[:, wc, :], h_cd[:, qt, wc * 128:(wc + 1) * 128], ident)
        nc.vector.tensor_copy(hT[:, :, qt * 128:(qt + 1) * 128], pst)

    hb2 = persist.tile([128, RT, W], f32)
    for qt in range(RT):
        nc.vector.tensor_add(hb2[:, qt, :], h_f32[:, qt, :], b2_rep)

    lT_all = persist.tile([128, FC, R], cd)
    for fc in range(FC):
        ps_y1 = stt([128, R])
        for wc in range(WC):
            mm(ps_y1, w1_all[:, fc, wc, :], hT[:, wc, :],
               start=(wc == 0), stop=(wc == WC - 1))
        nc.scalar.activation(lT_all[:, fc, :], ps_y1,
                             mybir.ActivationFunctionType.Prelu,
                             bias=b1_sb[:, fc:fc + 1], scale=1.0, alpha=LEAKY)

    out_re = t["out"].rearrange("(q p) w -> q p w", p=128)
    for qt in range(RT):
        ps_y2 = acc([128, W])
        for fc in range(FC):
            mm(ps_y2, lT_all[:, fc, qt * 128:(qt + 1) * 128],
               w2_all[:, fc, :], start=(fc == 0), stop=(fc == FC - 1))
        sum2 = stream.tile([128, W], f32, tag="sum")
        ssum = small.tile([128, 1], f32, tag="ssum")
        nc.vector.scalar_tensor_tensor(out=sum2, in0=ps_y2,
                                       scalar=1.0, in1=hb2[:, qt, :],
                                       op0=mybir.AluOpType.mult,
                                       op1=mybir.AluOpType.add,
                                       accum_out=ssum)
        ln_finish(sum2, sum2, ssum)
        nc.sync.dma_start(out=out_re[qt], in_=sum2)

    ctx.close()


def make_in_maps_general(inputs):
    import ml_dtypes
    wd = ml_dtypes.bfloat16

    def pm(a):
        c = a.shape[0] // 128
        return np.ascontiguousarray(
            a.reshape(c, 128, *a.shape[1:]).transpose(1, 0, 2), dtype=wd)

    f = {k: np.asarray(v, dtype=np.float32) for k, v in inputs.items()}
    shared = {
        "w_qm2": np.ascontiguousarray(
            np.stack([pm(f["w_q_m"]), pm(f["w_q_m"])], axis=2), dtype=wd),
        "w_qc2": np.ascontiguousarray(
            np.stack([pm(f["w_q_c"]), pm(f["w_q_c"])], axis=2), dtype=wd),
        "w_k2": np.ascontiguousarray(
            np.stack([pm(f["w_k_m"]), pm(f["w_k_c"])], axis=2), dtype=wd),
        "w_k2s": np.ascontiguousarray(
            np.stack([pm(f["w_k_c"]), pm(f["w_k_m"])], axis=2), dtype=wd),
        "w_v2": np.ascontiguousarray(
            np.stack([pm(f["w_v_m"]), pm(f["w_v_c"])], axis=2), dtype=wd),
        "w_o": np.ascontiguousarray(
            f["w_o"].reshape(H, P, W).transpose(1, 0, 2), dtype=wd),
        "ffn_w1": np.ascontiguousarray(
            f["ffn_w1"].reshape(WC, 128, FC, 128).transpose(1, 2, 0, 3), dtype=wd),
        "ffn_w2": np.ascontiguousarray(
            f["ffn_w2"].reshape(FC, 128, W).transpose(1, 0, 2), dtype=wd),
        "ffn_b1": np.ascontiguousarray(f["ffn_b1"].reshape(FC, 128).T),
        "ln_g": f["ln_g"], "ln_b": f["ln_b"], "ffn_b2": f["ffn_b2"],
    }
    x = f["x"]
    x_cd = x.astype(wd)
    shared["x_t"] = np.ascontiguousarray(x_cd.T)
    in_maps = []
    for c in range(NCORES):
        m = dict(shared)
        xr = x[c * R:(c + 1) * R]
        m["x_rows"] = np.ascontiguousarray(
            xr.reshape(RT, 128, W).transpose(1, 0, 2))
        m["xr_t"] = np.ascontiguousarray(
            x_cd.T[:, c * R:(c + 1) * R].reshape(WC, 128, R).transpose(1, 0, 2))
        in_maps.append(m)
    return in_maps


# ======================================================================
# dispatch
# ======================================================================

_NC_CACHE = {}


def build_nc(mode=MODE):
    return build_nc_fast() if mode == "fast" else build_nc_general()


def get_nc(mode=MODE):
    if mode not in _NC_CACHE:
        nc = build_nc(mode)
        nc.finalize()
        _NC_CACHE[mode] = nc
    return _NC_CACHE[mode]


def make_in_maps(inputs, mode=MODE):
    if mode == "fast":
        return make_in_maps_fast(inputs)
    return make_in_maps_general(inputs)


def pick_mode(inputs):
    if MODE != "fast":
        return MODE
    # fast path assumes the additive LN/FFN biases are zero (true for this
    # problem's setup_inputs); fall back to the general kernel otherwise.
    for k in ("ln_b", "ffn_b1", "ffn_b2"):
        v = np.asarray(inputs[k])
        if np.any(v != 0):
            return "general"
    return "fast"


def kernel(**inputs):
    mode = pick_mode(inputs)
    in_maps = make_in_maps(inputs, mode)
    nc = get_nc(mode)
    res = run_bass_kernel_spmd(nc, in_maps, list(range(NCORES)))
    return np.concatenate([res.results[c]["out"] for c in range(NCORES)], axis=0)


# revision 39
# speedup vs baseline: 1.1259x; 1.1259x over previous
"""Trainium2 Bass kernel for a small decoder block (nn_Decoder_75849122448079).

Math (N=4096 seq, W=512 width, P=64 proj, H=8 heads, F=2048 ffn):
  masked_mh = softmax(q_m k_m^T / 8) v_m @ w_o_sum      (w_o_sum = sum of H row-blocks of w_o)
  mh        = softmax(q_c k_c^T / 8) v_c @ w_o_sum      (q_c from masked_mh; k_c/v_c from x)
  h   = LN(mh + x) * g + b
  y   = LeakyReLU(h @ w1 + b1) @ w2 + b2
  out = LN(y + h) * g + b

Sharding: data-parallel over sequence rows — each of the 8 cores owns 512 query
rows end-to-end; K/V projections for the full sequence are replicated on every
core (no collectives). The host only slices x and re-lays-out / dtype-casts
weights (pure marshalling).

Fast path (requires ln_b == ffn_b1 == ffn_b2 == 0, which setup_inputs
guarantees; ln_g is handled generally): the middle LayerNorm's 1/std factor r
cancels exactly —
    y + h = r * (z2 + vc*g),  z2 = LeakyReLU(vc @ (g*w1)) @ w2,  vc = v - mean(v)
because LeakyReLU is positively homogeneous and LN is scale-invariant per row.
So no h, no middle LN, and no [q,w]->[w,q] transposes of h are ever computed.
Mean-centering rides as a rank-1 augmented row through the wosum matmuls, and
masked_mh is never materialized: q_c = A_mn @ (v_m @ wosum @ w_q_c) via the
precomputed 64x64 fold woq.
"""

import os

import numpy as np

import concourse.bass as bass
import concourse.bacc as bacc
import concourse.mybir as mybir
import concourse.tile as tile
from concourse.bass_utils import run_bass_kernel_spmd
from concourse.masks import make_identity

N, W, P, H, F = 4096, 512, 64, 8, 2048
# kt owning PE rows 0:64 / 64:128 of score group g (see K^T packing)
KT_TOP = [kt for sg in range(0, 32 // 4, 2) for kt in range(4 * sg, 4 * sg + 4)]
KT_BOT = [kt for sg in range(1, 32 // 4, 2) for kt in range(4 * sg, 4 * sg + 4)]
NCORES = 8
R = N // NCORES          # 512 rows per core
RT = R // 128            # 4 row tiles per core
WC = W // 128            # 4 contraction chunks over width
ST = N // 128            # 32 sequence (key) tiles
FC = F // 128            # 16 ffn-hidden tiles
NSG = 8                  # x^T DMA chunks (512 tokens each)
EPS = 1e-5
LEAKY = 0.01
SCALE = 0.125            # 1/sqrt(P)

f32 = mybir.dt.float32
bf16 = mybir.dt.bfloat16

MODE = os.environ.get("BASS_DECODER_MODE", "fast")


def _row_bcast(ap, parts=128):
    """AP reading a 1-D DRAM tensor replicated across `parts` partitions."""
    a = ap[:]
    return bass.AP(tensor=a.tensor, offset=a.offset, ap=[[0, parts]] + list(a.ap))


# ======================================================================
# fast path
# ======================================================================

def build_nc_fast():
    cd = bf16
    nc = bacc.Bacc()

    spec = [("x_rows", [128, RT, W], f32),
            ("x_t", [128, NSG, WC, 512], cd),
            ("xr_t", [128, WC, R], cd),
            # packed qkv weights: [w_k2 | w_k2s | w_v2 | w_qm2], each [WC, 2, P]
            ("w_qkv", [128, 4, WC, 2, P], cd),
            ("w_qc", [128, WC, P], cd),
            ("w_oT", [128, WC, W], cd),        # w_o^T, partition-major
            ("g_t", [128, WC], f32),           # ln_g partition-major
            ("ffn_w1", [128, FC, WC, 128], cd),
            ("ffn_w2", [128, FC, W], cd),
            ("ln_g", [W], f32), ("ln_b", [W], f32)]
    t = {}
    for n, s, d in spec:
        t[n] = nc.declare_dram_parameter(n, s, d, isOutput=False)
    t["out"] = nc.declare_dram_parameter("out", [R, W], f32, isOutput=True)

    with tile.TileContext(nc) as tc:
        _build_fast(tc, cd, t)
    return nc


def _build_fast(tc, cd, t):
    nc = tc.nc
    mm = nc.tensor.matmul

    def tp(out, in_, ident):  # PE transpose
        mm(out, in_, ident, is_transpose=True)

    from contextlib import ExitStack
    ctx = ExitStack()
    persist = ctx.enter_context(tc.tile_pool(name="persist", bufs=1))
    stream = ctx.enter_context(tc.tile_pool(name="stream", bufs=2))
    small = ctx.enter_context(tc.tile_pool(name="small", bufs=4))
    pt_pool = ctx.enter_context(tc.tile_pool(name="pt_pool", bufs=3))
    dram = ctx.enter_context(tc.tile_pool(name="dram", bufs=1, space="DRAM"))
    ps_kv = ctx.enter_context(tc.tile_pool(name="ps_kv", bufs=2, space="PSUM"))
    ps_st = ctx.enter_context(tc.tile_pool(name="ps_st", bufs=2, space="PSUM"))
    ps_ac = ctx.enter_context(tc.tile_pool(name="ps_ac", bufs=2, space="PSUM"))

    def big(shape, dtype=f32):        # 1-bank scratch (<=2KB/partition)
        return ps_kv.tile(shape, dtype, tag="kv", name="kvtile")

    def stt_ps(shape, dtype=f32):     # 2-bank score/ffn tiles
        return ps_st.tile(shape, dtype, tag="sT", name="sttile")

    def acc(shape, dtype=f32):        # 1-bank accumulators (aT, y2)
        return ps_ac.tile(shape, dtype, tag="acc", name="acctile")

    # ---------------- critical-path loads (SP queue): qkv weights, x^T ------
    wqkv = persist.tile([128, 4, WC, 2, P], cd)
    nc.sync.dma_start(out=wqkv, in_=t["w_qkv"][:])
    xrT = persist.tile([128, WC, R], cd)
    nc.scalar.dma_start(out=xrT, in_=t["xr_t"][:])
    xT = persist.tile([128, WC, N], cd)
    woT = persist.tile([128, WC, W], cd)

    def xchunk(sg, q):
        q.dma_start(out=xT[:, :, sg * 512:(sg + 1) * 512],
                    in_=t["x_t"][:, sg, :, :])
    xchunk(0, nc.sync)
    xchunk(1, nc.scalar)
    xchunk(2, nc.sync)
    xchunk(3, nc.scalar)
    nc.scalar.dma_start(out=woT, in_=t["w_oT"][:])
    xchunk(4, nc.sync)
    xchunk(5, nc.scalar)
    xchunk(6, nc.sync)
    xchunk(7, nc.scalar)

    # ---------------- small constants / secondary loads on the ACT queue ----
    ident = persist.tile([128, 128], cd)
    make_identity(nc, ident)
    ident_f32 = persist.tile([128, 128], f32)
    make_identity(nc, ident_f32)

    eps_t = persist.tile([128, 1], f32)
    nc.vector.memset(eps_t, EPS)
    zero_t = persist.tile([128, 1], f32)
    nc.vector.memset(zero_t, 0.0)

    wqc = persist.tile([128, WC, P], cd)
    nc.scalar.dma_start(out=wqc, in_=t["w_qc"][:])
    g_rep = persist.tile([128, W], f32)
    nc.scalar.dma_start(out=g_rep, in_=_row_bcast(t["ln_g"]))
    b_rep = persist.tile([128, W], f32)
    nc.scalar.dma_start(out=b_rep, in_=_row_bcast(t["ln_b"]))
    g_t = persist.tile([128, WC], f32)
    nc.scalar.dma_start(out=g_t, in_=t["g_t"][:])

    # K^T / V packed for the attention loops
    G = ST // 2
    kmT = persist.tile([128, G, 128], cd)
    kcT = persist.tile([128, G, 128], cd)
    vm = persist.tile([128, ST, P + 1], cd)
    vc = persist.tile([128, ST, P + 1], cd)
    nc.vector.memset(vm[:, :, P:P + 1], 1.0)
    nc.vector.memset(vc[:, :, P:P + 1], 1.0)

    # PE warm-up: ~3.5us of dummy matmuls while the DMAs land, so the HAM
    # clock gate opens (1.2 -> 2.4 GHz) before the real projections start.
    # f32 runs at 4 cycles/row, so 8 matmuls of 128 columns ~= 3.4us cold.
    ps_warm = big([128, 128])
    for i in range(8):
        mm(ps_warm, ident_f32, ident_f32, start=(i == 0), stop=(i == 7),
           skip_group_check=True)

    # ---------------- Q^T (masked), duplicated into both partition halves ---
    ps_q = big([128, R])
    for wc in range(WC):
        mm(ps_q, wqkv[:, 3, wc, :, :], xrT[:, wc, :], start=(wc == 0), stop=(wc == WC - 1))
    qmT = persist.tile([128, R], cd)
    qm_copy = nc.vector.tensor_copy(qmT, ps_q)

    # ------- replicated K/V projections over the full sequence.  All PSUM
    # evacuation copies go on the Vector engine so the Scalar engine is kept
    # free for the attention exps that overlap this phase.
    last_proj_copy = [None]

    def proj_sgs(sg_lo, sg_hi):
        for sg in range(sg_lo, sg_hi):
            ps_k = big([128, 512])
            wki = 0 if sg % 2 == 0 else 1
            for wc in range(WC):
                mm(ps_k, wqkv[:, wki, wc, :, :], xT[:, wc, sg * 512:(sg + 1) * 512],
                   start=(wc == 0), stop=(wc == WC - 1))
            lo, hi = 4 * (sg // 2), 4 * (sg // 2) + 4
            if sg % 2 == 0:  # top rows = km, bottom = kc
                nc.vector.tensor_copy(kmT[0:64, lo:hi, :], ps_k[0:64, :])
                nc.vector.tensor_copy(kcT[64:128, lo:hi, :], ps_k[64:128, :])
            else:            # top rows = kc, bottom = km
                nc.vector.tensor_copy(kcT[0:64, lo:hi, :], ps_k[0:64, :])
                nc.vector.tensor_copy(kmT[64:128, lo:hi, :], ps_k[64:128, :])
            # all 16 MMs form ONE accumulation group in one PSUM bank: start
            # clears the whole bank once, then each byte is overwritten on its
            # first write and accumulated after (order-independent).
            ps_v4 = big([128, 4, 2, P])
            for st4 in range(4):
                st = 4 * sg + st4
                for wc in range(WC):
                    mm(ps_v4[:, st4, :, :], xT[:, wc, st * 128:(st + 1) * 128],
                       wqkv[:, 2, wc, :, :],
                       start=(st4 == 0 and wc == 0), stop=(st4 == 3 and wc == WC - 1),
                       skip_group_check=True)
            nc.vector.tensor_copy(vm[:, 4 * sg:4 * sg + 4, 0:P], ps_v4[:, :, 0, :])
            last_proj_copy[0] = nc.vector.tensor_copy(
                vc[:, 4 * sg:4 * sg + 4, 0:P], ps_v4[:, :, 1, :])

    proj_sgs(0, 4)

    # ---- w_o-derived constants: emitted between the projection halves so
    # they are ready-compatible (woT lands ~19us) and their DVE ops never
    # block later projection copies or the masked-post chain.
    woTs_f32 = persist.tile([128, WC, P], f32)
    nc.vector.tensor_add(woTs_f32, woT[:, :, 0:P], woT[:, :, P:2 * P])
    for hh in range(2, H):
        nc.vector.tensor_add(woTs_f32, woTs_f32, woT[:, :, hh * P:(hh + 1) * P])
    woTs = persist.tile([128, WC, P], cd)
    nc.vector.tensor_copy(woTs, woTs_f32)

    wosum_aug = persist.tile([P + 1, W], cd)
    nc.vector.memset(wosum_aug[P:P + 1, :], 1.0)
    for wc in range(WC):
        ps_wo = big([P, 128], cd)
        tp(ps_wo, woTs[:, wc, :], ident)
        nc.vector.tensor_copy(wosum_aug[0:P, wc * 128:(wc + 1) * 128], ps_wo)

    # woq = wosum @ w_q_c  [64, 64], then duplicated -> [64, 128]
    ps_woq = big([P, P])
    for wc in range(WC):
        mm(ps_woq, woTs[:, wc, :], wqc[:, wc, :],
           start=(wc == 0), stop=(wc == WC - 1))
    woq_dup = persist.tile([P, 128], cd)
    nc.vector.tensor_copy(woq_dup[:, 0:P], ps_woq)
    nc.vector.tensor_copy(woq_dup[:, P:2 * P], ps_woq)

    proj_sgs(4, 8)

    # ------------------------------------------------------------- attention
    def scores_pair(kT, qT, g):
        sT = stt_ps([128, 2, 512])
        mm(sT[:, 0, :], kT[0:64, g, :], qT[0:64, :], tile_position=(0, 0))
        mm(sT[:, 1, :], kT[64:128, g, :], qT[64:128, :], tile_position=(64, 0))
        return sT

    def attn_run(kT, v, qT, ps_aT, g_lo, g_hi, first, last):
        """Score/exp/accumulate groups [g_lo, g_hi) with one-group lookahead."""
        sT_prev = scores_pair(kT, qT, g_lo)
        for g in range(g_lo + 1, g_hi + 1):
            sT_next = scores_pair(kT, qT, g) if g < g_hi else None
            ptl = pt_pool.tile([128, 2, 512], cd, tag="pt")
            nc.scalar.activation(ptl, sT_prev, mybir.ActivationFunctionType.Exp,
                                 scale=SCALE)
            for j in range(2):
                kt = (KT_TOP, KT_BOT)[j][g - 1]
                mm(ps_aT, v[:, kt, :], ptl[:, j, :],
                   start=(first and g == g_lo + 1 and j == 0),
                   stop=(last and g == g_hi and j == 1))
            sT_prev = sT_next

    # masked attention: all 32 key tiles
    ps_aTm = acc([P + 1, R])
    attn_run(kmT, vm, qmT, ps_aTm, 0, G, True, True)

    amT = persist.tile([P + 1, R], f32, tag="amT", name="amT")
    am_copy = nc.vector.tensor_copy(amT, ps_aTm)

    # ---------------- masked -> cross: normalize A, q_c^T = woq^T @ A_mn^T --
    # normalize mult + evacuations run on the (idle) Scalar engine so the DVE
    # and PE ping-pong stays short.
    amT_n = persist.tile([P, R], cd)
    for qt in range(RT):
        ps_a4 = big([128, P + 1])
        tp(ps_a4, amT[:, qt * 128:(qt + 1) * 128], ident_f32[0:P + 1, 0:P + 1])
        recip_m = small.tile([128, 1], f32, tag="recip")
        nc.vector.reciprocal(recip_m, ps_a4[:, P:P + 1])
        a_m = small.tile([128, P], cd, tag="a_m")
        nc.scalar.activation(a_m, ps_a4[:, 0:P],
                             mybir.ActivationFunctionType.Copy, scale=recip_m)
        ps_at2 = big([P, 128], cd)
        tp(ps_at2, a_m, ident)
        nc.vector.tensor_copy(amT_n[:, qt * 128:(qt + 1) * 128], ps_at2)

    ps_qc = stt_ps([128, R])
    mm(ps_qc, woq_dup, amT_n)
    qcT = persist.tile([128, R], cd)
    qc_copy = nc.vector.tensor_copy(qcT, ps_qc)

    # ---- bulk loads: anchored to the end of the projections, so they use
    # the idle DMA window during masked attention.
    from concourse.bass import _add_dep_helper
    anchor = last_proj_copy[0]
    xr_nat = persist.tile([128, RT, W], f32)
    d0 = nc.scalar.dma_start(out=xr_nat, in_=t["x_rows"][:])
    _add_dep_helper(d0.ins, anchor.ins, sync=True, reason="delay x_rows load")
    w1_all = persist.tile([128, FC, WC, 128], cd)
    d1 = nc.scalar.dma_start(out=w1_all, in_=t["ffn_w1"][:])
    _add_dep_helper(d1.ins, anchor.ins, sync=True, reason="delay ffn w1 preload")
    w2_all = persist.tile([128, FC, W], cd)
    d2 = nc.scalar.dma_start(out=w2_all, in_=t["ffn_w2"][:])
    _add_dep_helper(d2.ins, anchor.ins, sync=True, reason="delay ffn w2 preload")

    # wosumg_aug = wosum_aug * g[w]  (g folded for the residual path)
    wosumg_aug = persist.tile([P + 1, W], cd)
    nc.vector.tensor_mul(wosumg_aug, wosum_aug, g_rep[0:P + 1, :])
    # wsr = sum_w wosum[d, w] -> DRAM roundtrip -> replicated [128, 64]
    wsr = small.tile([P, 1], f32, tag="wsr")
    junk = stream.tile([P, W], cd, tag="junk")
    nc.vector.scalar_tensor_tensor(out=junk, in0=wosum_aug[0:P, :], scalar=1.0,
                                   in1=wosum_aug[0:P, :],
                                   op0=mybir.AluOpType.mult,
                                   op1=mybir.AluOpType.max, accum_out=wsr)
    wsr_dram = dram.tile([P], f32)
    nc.scalar.dma_start(out=wsr_dram, in_=wsr[:, 0])
    wsr_rep = persist.tile([128, P], f32)
    nc.scalar.dma_start(out=wsr_rep, in_=_row_bcast(wsr_dram))

    # ------------------------------------------------------------------ cross
    ps_aTc = acc([P + 1, R])
    attn_run(kcT, vc, qcT, ps_aTc, 0, G, True, True)

    # ---- x-derived prep: runs on the DVE during cross attention.  Each op
    # carries an explicit dependency on the masked-post chain so the static
    # scheduler can never slot it ahead of that critical path.
    xsum = persist.tile([128, RT], f32)
    for qt in range(RT):
        xs_scr = stream.tile([128, W], f32, tag="junk2")
        i0 = nc.vector.scalar_tensor_tensor(out=xs_scr, in0=xr_nat[:, qt, :],
                                            scalar=1.0, in1=xr_nat[:, qt, :],
                                            op0=mybir.AluOpType.mult,
                                            op1=mybir.AluOpType.max,
                                            accum_out=xsum[:, qt:qt + 1])
        _add_dep_helper(i0.ins, qc_copy.ins, sync=True, reason="after masked-post")
    # xg = x * g (residual path), and fold ln_g into ffn_w1 in place
    xg_nat = persist.tile([128, RT, W], f32)
    for qt in range(RT):
        i0 = nc.vector.tensor_mul(xg_nat[:, qt, :], xr_nat[:, qt, :], g_rep)
        _add_dep_helper(i0.ins, qc_copy.ins, sync=True, reason="after masked-post")
    for wc in range(WC):
        i0 = nc.vector.tensor_scalar_mul(w1_all[:, :, wc, :],
                                         w1_all[:, :, wc, :],
                                         g_t[:, wc:wc + 1])
        _add_dep_helper(i0.ins, qc_copy.ins, sync=True, reason="after masked-post")

    acT = persist.tile([P + 1, R], f32, tag="acT", name="acT")
    nc.vector.tensor_copy(acT, ps_aTc)

    # ------- cross post: normalize + augment with the -mean row, transposed
    # acn_aug[0:64, q] = A_cn^T ; acn_aug[64, q] = -mean_w(v[q, :])
    acn_aug = persist.tile([P + 1, R], cd)
    for qt in range(RT):
        ps_a4 = big([128, P + 1])
        tp(ps_a4, acT[:, qt * 128:(qt + 1) * 128], ident_f32[0:P + 1, 0:P + 1])
        recip_c = small.tile([128, 1], f32, tag="recip")
        nc.vector.reciprocal(recip_c, ps_a4[:, P:P + 1])
        a_n = small.tile([128, P + 1], cd, tag="a_n")
        nc.scalar.activation(a_n[:, 0:P], ps_a4[:, 0:P],
                             mybir.ActivationFunctionType.Copy, scale=recip_c)
        # t1[q] = (A_c[q, :] . wsr), normalized inside the aug combine below
        t1 = small.tile([128, 1], f32, tag="t1")
        scr = stream.tile([128, P], f32, tag="mscr")
        nc.vector.scalar_tensor_tensor(out=scr, in0=ps_a4[:, 0:P], scalar=1.0,
                                       in1=wsr_rep, op0=mybir.AluOpType.mult,
                                       op1=mybir.AluOpType.mult, accum_out=t1)
        # aug col = -(t1 * recip + xsum)/W
        t2 = small.tile([128, 1], f32, tag="t2")
        nc.vector.scalar_tensor_tensor(out=t2, in0=t1, scalar=recip_c,
                                       in1=xsum[:, qt:qt + 1],
                                       op0=mybir.AluOpType.mult,
                                       op1=mybir.AluOpType.add)
        nc.vector.tensor_scalar_mul(a_n[:, P:P + 1], t2, -1.0 / W)
        ps_at2 = big([P + 1, 128], cd)
        tp(ps_at2, a_n, ident)
        nc.scalar.copy(acn_aug[:, qt * 128:(qt + 1) * 128], ps_at2)

    # vc^T[w, q] = (wosum_aug^T @ acn_aug) + x^T   (centered v, transposed).
    # The x^T term rides as an identity-matmul accumulate; evacuation goes on
    # the Scalar engine, which is idle between the last exp and first Prelu.
    vcT = persist.tile([128, WC, R], cd)
    for wc in range(WC):
        ps_v1 = stt_ps([128, R])
        mm(ps_v1, wosum_aug[:, wc * 128:(wc + 1) * 128], acn_aug,
           start=True, stop=False, skip_group_check=True)
        mm(ps_v1, ident, xrT[:, wc, :], start=False, stop=True,
           skip_group_check=True)
        nc.scalar.copy(vcT[:, wc, :], ps_v1)

    # ------------------------------------------------------------------- FFN
    luT = persist.tile([128, FC, R], cd)
    for fc in range(FC):
        ps_y1 = stt_ps([128, R])
        for wc in range(WC):
            mm(ps_y1, w1_all[:, fc, wc, :], vcT[:, wc, :],
               start=(wc == 0), stop=(wc == WC - 1))
        nc.scalar.activation(luT[:, fc, :], ps_y1,
                             mybir.ActivationFunctionType.Prelu,
                             bias=zero_t, scale=1.0, alpha=LEAKY)

    # vcg[q, w] = (v - mean) * g  (residual for the final LN; not needed until
    # FFN2, so it is emitted after FFN1 to fill PE gaps there)
    vcg = persist.tile([128, RT, W], f32)
    for qt in range(RT):
        ps_vq = big([128, W])
        mm(ps_vq, acn_aug[:, qt * 128:(qt + 1) * 128], wosumg_aug)
        nc.vector.scalar_tensor_tensor(out=vcg[:, qt, :], in0=ps_vq, scalar=1.0,
                                       in1=xg_nat[:, qt, :],
                                       op0=mybir.AluOpType.mult,
                                       op1=mybir.AluOpType.add)

    def ln_finish(dst, v_sb, ssum):
        """dst = LN(v_sb) * g + b, with sum(v) already in ssum [128, 1]."""
        scr = stream.tile([128, W], f32, tag="scr")
        ss2 = small.tile([128, 1], f32, tag="ss2")
        nc.scalar.activation(scr, v_sb, mybir.ActivationFunctionType.Square,
                             accum_out=ss2)
        m = small.tile([128, 1], f32, tag="m")
        nc.vector.tensor_scalar_mul(m, ssum, 1.0 / W)
        var = small.tile([128, 1], f32, tag="var")
        nc.vector.tensor_mul(var, m, m)
        nc.vector.scalar_tensor_tensor(out=var, in0=ss2, scalar=1.0 / W,
                                       in1=var, op0=mybir.AluOpType.mult,
                                       op1=mybir.AluOpType.subtract)
        nc.scalar.activation(var, var, mybir.ActivationFunctionType.Sqrt,
                             bias=eps_t, scale=1.0)
        nc.vector.reciprocal(var, var)
        nc.vector.tensor_scalar(dst, v_sb, scalar1=m, scalar2=var,
                                op0=mybir.AluOpType.subtract,
                                op1=mybir.AluOpType.mult)
        nc.vector.tensor_mul(dst, dst, g_rep)
        nc.vector.tensor_add(dst, dst, b_rep)

    out_re = t["out"].rearrange("(q p) w -> q p w", p=128)
    for qt in range(RT):
        ps_y2 = acc([128, W])
        for fc in range(FC):
            mm(ps_y2, luT[:, fc, qt * 128:(qt + 1) * 128],
               w2_all[:, fc, :], start=(fc == 0), stop=(fc == FC - 1))
        sum2 = stream.tile([128, W], f32, tag="sum")
        ssum = small.tile([128, 1], f32, tag="ssum")
        nc.vector.scalar_tensor_tensor(out=sum2, in0=ps_y2,
                                       scalar=1.0, in1=vcg[:, qt, :],
                                       op0=mybir.AluOpType.mult,
                                       op1=mybir.AluOpType.add,
                                       accum_out=ssum)
        ln_finish(sum2, sum2, ssum)
        nc.sync.dma_start(out=out_re[qt], in_=sum2)

    ctx.close()


def make_in_maps_fast(inputs):
    import ml_dtypes
    wd = ml_dtypes.bfloat16

    def pm(a):  # [(c p), d] -> [p, c, d]  (partition-major for contiguous DMA)
        c = a.shape[0] // 128
        return np.ascontiguousarray(
            a.reshape(c, 128, *a.shape[1:]).transpose(1, 0, 2), dtype=wd)

    f = {k: np.asarray(v, dtype=np.float32) for k, v in inputs.items()}
    x = f["x"]
    x_cd = x.astype(wd)
    xt = x_cd.T  # [W, N]
    w_k2 = np.stack([pm(f["w_k_m"]), pm(f["w_k_c"])], axis=2)
    w_k2s = np.stack([pm(f["w_k_c"]), pm(f["w_k_m"])], axis=2)
    w_v2 = np.stack([pm(f["w_v_m"]), pm(f["w_v_c"])], axis=2)
    w_qm2 = np.stack([pm(f["w_q_m"]), pm(f["w_q_m"])], axis=2)
    shared = {
        # one packed DMA: [w_k2 | w_k2s | w_v2 | w_qm2]
        "w_qkv": np.ascontiguousarray(
            np.stack([w_k2, w_k2s, w_v2, w_qm2], axis=1), dtype=wd),
        "w_qc": pm(f["w_q_c"]),
        # w_o^T [(c p), (h d)] -> [p, c, (h d)]
        "w_oT": np.ascontiguousarray(
            f["w_o"].T.reshape(WC, 128, H * P).transpose(1, 0, 2), dtype=wd),
        "g_t": np.ascontiguousarray(f["ln_g"].reshape(WC, 128).T,
                                    dtype=np.float32),
        # x^T chunk-contiguous: [p, sg, c, j] = x[sg*512+j, c*128+p]
        "x_t": np.ascontiguousarray(
            xt.reshape(WC, 128, NSG, 512).transpose(1, 2, 0, 3), dtype=wd),
        "ffn_w1": np.ascontiguousarray(
            f["ffn_w1"].reshape(WC, 128, FC, 128).transpose(1, 2, 0, 3), dtype=wd),
        "ffn_w2": np.ascontiguousarray(
            f["ffn_w2"].reshape(FC, 128, W).transpose(1, 0, 2), dtype=wd),
        "ln_g": f["ln_g"], "ln_b": f["ln_b"],
    }
    in_maps = []
    for c in range(NCORES):
        m = dict(shared)
        xr = x[c * R:(c + 1) * R]  # [R, W] -> [p, q, w]
        m["x_rows"] = np.ascontiguousarray(
            xr.reshape(RT, 128, W).transpose(1, 0, 2))
        m["xr_t"] = np.ascontiguousarray(
            x_cd.T[:, c * R:(c + 1) * R].reshape(WC, 128, R).transpose(1, 0, 2))
        in_maps.append(m)
    return in_maps


# ======================================================================
# general fallback path (handles nonzero ln_b / ffn_b1 / ffn_b2)
# ======================================================================

def build_nc_general():
    cd = bf16
    nc = bacc.Bacc()

    spec = [("x_rows", [128, RT, W], f32),
            ("x_t", [W, N], cd),
            ("xr_t", [128, WC, R], cd),
            ("w_qm2", [128, WC, 2, P], cd),
            ("w_qc2", [128, WC, 2, P], cd),
            ("w_k2", [128, WC, 2, P], cd),
            ("w_k2s", [128, WC, 2, P], cd),
            ("w_v2", [128, WC, 2, P], cd),
            ("w_o", [64, H, W], cd),
            ("ffn_w1", [128, FC, WC, 128], cd),
            ("ffn_w2", [128, FC, W], cd),
            ("ln_g", [W], f32), ("ln_b", [W], f32),
            ("ffn_b1", [128, FC], f32), ("ffn_b2", [W], f32)]
    t = {}
    for n, s, d in spec:
        t[n] = nc.declare_dram_parameter(n, s, d, isOutput=False)
    t["out"] = nc.declare_dram_parameter("out", [R, W], f32, isOutput=True)

    with tile.TileContext(nc) as tc:
        _build_general(tc, cd, t)
    return nc


def _build_general(tc, cd, t):
    nc = tc.nc
    mm = nc.tensor.matmul

    def tp(out, in_, ident):  # PE transpose
        mm(out, in_, ident, is_transpose=True)

    from contextlib import ExitStack
    ctx = ExitStack()
    persist = ctx.enter_context(tc.tile_pool(name="persist", bufs=1))
    stream = ctx.enter_context(tc.tile_pool(name="stream", bufs=2))
    wstream = ctx.enter_context(tc.tile_pool(name="wstream", bufs=3))
    small = ctx.enter_context(tc.tile_pool(name="small", bufs=4))
    pt_pool = ctx.enter_context(tc.tile_pool(name="pt_pool", bufs=3))
    dram = ctx.enter_context(tc.tile_pool(name="dram", bufs=1, space="DRAM"))
    ps_kv = ctx.enter_context(tc.tile_pool(name="ps_kv", bufs=2, space="PSUM"))
    ps_st = ctx.enter_context(tc.tile_pool(name="ps_st", bufs=2, space="PSUM"))
    ps_ac = ctx.enter_context(tc.tile_pool(name="ps_ac", bufs=2, space="PSUM"))

    def big(shape, dtype=f32):
        return ps_kv.tile(shape, dtype, tag="kv", name="kvtile")

    def stt(shape, dtype=f32):
        return ps_st.tile(shape, dtype, tag="sT", name="sttile")

    def acc(shape, dtype=f32):
        return ps_ac.tile(shape, dtype, tag="acc", name="acctile")

    wqm2 = persist.tile([128, WC, 2, P], cd)
    nc.sync.dma_start(out=wqm2, in_=t["w_qm2"][:])
    wqc2 = persist.tile([128, WC, 2, P], cd)
    nc.sync.dma_start(out=wqc2, in_=t["w_qc2"][:])
    wk2 = persist.tile([128, WC, 2, P], cd)
    nc.sync.dma_start(out=wk2, in_=t["w_k2"][:])
    wk2s = persist.tile([128, WC, 2, P], cd)
    nc.sync.dma_start(out=wk2s, in_=t["w_k2s"][:])
    wv2 = persist.tile([128, WC, 2, P], cd)
    nc.sync.dma_start(out=wv2, in_=t["w_v2"][:])
    xrT = persist.tile([128, WC, R], cd)
    nc.sync.dma_start(out=xrT, in_=t["xr_t"][:])
    xT = persist.tile([128, WC, N], cd)
    x_t_re = t["x_t"].rearrange("(c p) n -> p c n", p=128)
    for sg in range(NSG):
        nc.sync.dma_start(out=xT[:, :, sg * (N // NSG):(sg + 1) * (N // NSG)],
                          in_=x_t_re[:, :, sg * (N // NSG):(sg + 1) * (N // NSG)])

    ident = persist.tile([128, 128], cd)
    make_identity(nc, ident)
    ident_f32 = persist.tile([128, 128], f32)
    make_identity(nc, ident_f32)

    eps_t = persist.tile([128, 1], f32)
    nc.vector.memset(eps_t, EPS)

    g_rep = persist.tile([128, W], f32)
    nc.scalar.dma_start(out=g_rep, in_=_row_bcast(t["ln_g"]))
    b_rep = persist.tile([128, W], f32)
    nc.scalar.dma_start(out=b_rep, in_=_row_bcast(t["ln_b"]))
    b2_rep = persist.tile([128, W], f32)
    nc.scalar.dma_start(out=b2_rep, in_=_row_bcast(t["ffn_b2"]))
    b1_sb = persist.tile([128, FC], f32)
    nc.scalar.dma_start(out=b1_sb, in_=t["ffn_b1"][:])

    wo_stage = stream.tile([64, H, W], cd, tag="wo")
    nc.scalar.dma_start(out=wo_stage, in_=t["w_o"][:])
    wos_f32 = persist.tile([64, W], f32)
    nc.vector.tensor_add(wos_f32, wo_stage[:, 0, :], wo_stage[:, 1, :])
    for hh in range(2, H):
        nc.vector.tensor_add(wos_f32, wos_f32, wo_stage[:, hh, :])
    wosum = persist.tile([64, W], cd)
    nc.vector.tensor_copy(wosum, wos_f32)

    xr_nat = persist.tile([128, RT, W], f32)
    nc.scalar.dma_start(out=xr_nat, in_=t["x_rows"][:])

    G = ST // 2
    kmT = persist.tile([128, G, 128], cd)
    kcT = persist.tile([128, G, 128], cd)
    vm = persist.tile([128, ST, P + 1], cd)
    vc = persist.tile([128, ST, P + 1], cd)
    nc.vector.memset(vm[:, :, P:P + 1], 1.0)
    nc.vector.memset(vc[:, :, P:P + 1], 1.0)

    ps_q = big([128, R])
    for wc in range(WC):
        mm(ps_q, wqm2[:, wc, :, :], xrT[:, wc, :], start=(wc == 0), stop=(wc == WC - 1))
    qmT = persist.tile([128, R], cd)
    qm_copy = nc.vector.tensor_copy(qmT, ps_q)

    def proj_sgs(sg_lo, sg_hi):
        for sg in range(sg_lo, sg_hi):
            ps_k = big([128, 512])
            wk = wk2 if sg % 2 == 0 else wk2s
            for wc in range(WC):
                mm(ps_k, wk[:, wc, :, :], xT[:, wc, sg * 512:(sg + 1) * 512],
                   start=(wc == 0), stop=(wc == WC - 1))
            lo, hi = 4 * (sg // 2), 4 * (sg // 2) + 4
            if sg % 2 == 0:
                nc.scalar.copy(kmT[0:64, lo:hi, :], ps_k[0:64, :])
                nc.vector.tensor_copy(kcT[64:128, lo:hi, :], ps_k[64:128, :])
            else:
                nc.scalar.copy(kcT[0:64, lo:hi, :], ps_k[0:64, :])
                nc.vector.tensor_copy(kmT[64:128, lo:hi, :], ps_k[64:128, :])
            for st in range(4 * sg, 4 * sg + 4):
                ps_v = big([128, 2, P])
                for wc in range(WC):
                    mm(ps_v, xT[:, wc, st * 128:(st + 1) * 128], wv2[:, wc, :, :],
                       start=(wc == 0), stop=(wc == WC - 1))
                nc.scalar.copy(vm[:, st, 0:P], ps_v[:, 0, :])
                nc.vector.tensor_copy(vc[:, st, 0:P], ps_v[:, 1, :])

    from concourse.bass import _add_dep_helper
    w1_all = persist.tile([128, FC, WC, 128], cd)
    d1 = nc.scalar.dma_start(out=w1_all, in_=t["ffn_w1"][:])
    _add_dep_helper(d1.ins, qm_copy.ins, sync=True, reason="delay ffn w1 preload")
    w2_all = persist.tile([128, FC, W], cd)
    d2 = nc.scalar.dma_start(out=w2_all, in_=t["ffn_w2"][:])
    _add_dep_helper(d2.ins, qm_copy.ins, sync=True, reason="delay ffn w2 preload")

    def scores_pair(kT, qT, g):
        sT = stt([128, 2, 512])
        mm(sT[:, 0, :], kT[0:64, g, :], qT[0:64, :])
        mm(sT[:, 1, :], kT[64:128, g, :], qT[64:128, :])
        return sT

    def attn_run(kT, v, qT, ps_aT, g_lo, g_hi, first, last):
        sT_prev = scores_pair(kT, qT, g_lo)
        for g in range(g_lo + 1, g_hi + 1):
            sT_next = scores_pair(kT, qT, g) if g < g_hi else None
            ptl = pt_pool.tile([128, 2, 512], cd, tag="pt")
            nc.scalar.activation(ptl, sT_prev, mybir.ActivationFunctionType.Exp,
                                 scale=SCALE)
            for j in range(2):
                kt = (KT_TOP, KT_BOT)[j][g - 1]
                mm(ps_aT, v[:, kt, :], ptl[:, j, :],
                   start=(first and g == g_lo + 1 and j == 0),
                   stop=(last and g == g_hi and j == 1))
            sT_prev = sT_next

    ps_aTm = acc([P + 1, R])
    proj_sgs(0, 4)
    attn_run(kmT, vm, qmT, ps_aTm, 0, G // 2, True, False)
    proj_sgs(4, 8)
    attn_run(kmT, vm, qmT, ps_aTm, G // 2, G, False, True)
    amT = persist.tile([P + 1, R], f32, tag="amT", name="amT")
    nc.vector.tensor_copy(amT, ps_aTm)

    ps_a4 = big([128, RT, P + 1])
    for qt in range(RT):
        tp(ps_a4[:, qt, :], amT[:, qt * 128:(qt + 1) * 128],
           ident_f32[0:P + 1, 0:P + 1])
    a_m = small.tile([128, RT, P], cd, tag="a_m")
    recip_m = small.tile([128, RT, 1], f32, tag="recip")
    for qt in range(RT):
        nc.vector.reciprocal(recip_m[:, qt, :], ps_a4[:, qt, P:P + 1])
        nc.vector.tensor_scalar_mul(a_m[:, qt, :], ps_a4[:, qt, 0:P],
                                    recip_m[:, qt, :])
    ps_at2 = big([P, R], cd)
    for qt in range(RT):
        tp(ps_at2[:, qt * 128:(qt + 1) * 128], a_m[:, qt, :], ident)
    amT_n = persist.tile([P, R], cd)
    nc.vector.tensor_copy(amT_n, ps_at2)

    mhT = persist.tile([128, WC, R], cd)
    for wc in range(WC):
        ps_mh = stt([128, R])
        mm(ps_mh, wosum[:, wc * 128:(wc + 1) * 128], amT_n)
        nc.vector.tensor_copy(mhT[:, wc, :], ps_mh)

    ps_qc = big([128, R])
    for wc in range(WC):
        mm(ps_qc, wqc2[:, wc, :, :], mhT[:, wc, :], start=(wc == 0), stop=(wc == WC - 1))
    qcT = persist.tile([128, R], cd)
    nc.vector.tensor_copy(qcT, ps_qc)

    ps_aTc = acc([P + 1, R])
    attn_run(kcT, vc, qcT, ps_aTc, 0, G, True, True)
    acT = persist.tile([P + 1, R], f32, tag="acT", name="acT")
    nc.vector.tensor_copy(acT, ps_aTc)

    ps_s1 = big([128, RT, 1])
    for qt in range(RT):
        tp(ps_s1[:, qt, :], acT[P:P + 1, qt * 128:(qt + 1) * 128],
           ident_f32[P:P + 1, P:P + 1])
    rs_c = small.tile([128, RT, 1], f32, tag="rs_c")
    for qt in range(RT):
        nc.vector.reciprocal(rs_c[:, qt, :], ps_s1[:, qt, :])

    acT_cd = persist.tile([P + 1, R], cd)
    nc.vector.tensor_copy(acT_cd, acT)

    h_f32 = persist.tile([128, RT, W], f32)

    def ln_finish(dst, v_sb, ssum):
        scr = stream.tile([128, W], f32, tag="scr")
        ss2 = small.tile([128, 1], f32, tag="ss2")
        nc.scalar.activation(scr, v_sb, mybir.ActivationFunctionType.Square,
                             accum_out=ss2)
        m = small.tile([128, 1], f32, tag="m")
        nc.vector.tensor_scalar_mul(m, ssum, 1.0 / W)
        var = small.tile([128, 1], f32, tag="var")
        nc.vector.tensor_mul(var, m, m)
        nc.vector.scalar_tensor_tensor(out=var, in0=ss2, scalar=1.0 / W,
                                       in1=var, op0=mybir.AluOpType.mult,
                                       op1=mybir.AluOpType.subtract)
        nc.scalar.activation(var, var, mybir.ActivationFunctionType.Sqrt,
                             bias=eps_t, scale=1.0)
        nc.vector.reciprocal(var, var)
        nc.vector.tensor_scalar(dst, v_sb, scalar1=m, scalar2=var,
                                op0=mybir.AluOpType.subtract,
                                op1=mybir.AluOpType.mult)
        nc.vector.tensor_mul(dst, dst, g_rep)
        nc.vector.tensor_add(dst, dst, b_rep)

    for qt in range(RT):
        ps_mhc = stt([128, W])
        mm(ps_mhc, acT_cd[0:P, qt * 128:(qt + 1) * 128], wosum)
        sum_sb = stream.tile([128, W], f32, tag="sum")
        ssum = small.tile([128, 1], f32, tag="ssum")
        nc.vector.scalar_tensor_tensor(out=sum_sb, in0=ps_mhc,
                                       scalar=rs_c[:, qt, :],
                                       in1=xr_nat[:, qt, :],
                                       op0=mybir.AluOpType.mult,
                                       op1=mybir.AluOpType.add,
                                       accum_out=ssum)
        ln_finish(h_f32[:, qt, :], sum_sb, ssum)

    h_cd = persist.tile([128, RT, W], cd)
    nc.vector.tensor_copy(h_cd, h_f32)

    hT = persist.tile([128, WC, R], cd)
    for qt in range(RT):
        pst = big([128, WC, 128], cd)
        for wc in range(WC):
            tp(pst[:, wc, :], h_cd[:, qt, wc * 128:(wc + 1) * 128], ident)
        nc.vector.tensor_copy(hT[:, :, qt * 128:(qt + 1) * 128], pst)

    hb2 = persist.tile([128, RT, W], f32)
    for qt in range(RT):
        nc.vector.tensor_add(hb2[:, qt, :], h_f32[:, qt, :], b2_rep)

    lT_all = persist.tile([128, FC, R], cd)
    for fc in range(FC):
        ps_y1 = stt([128, R])
        for wc in range(WC):
            mm(ps_y1, w1_all[:, fc, wc, :], hT[:, wc, :],
               start=(wc == 0), stop=(wc == WC - 1))
        nc.scalar.activation(lT_all[:, fc, :], ps_y1,
                             mybir.ActivationFunctionType.Prelu,
                             bias=b1_sb[:, fc:fc + 1], scale=1.0, alpha=LEAKY)

    out_re = t["out"].rearrange("(q p) w -> q p w", p=128)
    for qt in range(RT):
        ps_y2 = acc([128, W])
        for fc in range(FC):
            mm(ps_y2, lT_all[:, fc, qt * 128:(qt + 1) * 128],
               w2_all[:, fc, :], start=(fc == 0), stop=(fc == FC - 1))
        sum2 = stream.tile([128, W], f32, tag="sum")
        ssum = small.tile([128, 1], f32, tag="ssum")
        nc.vector.scalar_tensor_tensor(out=sum2, in0=ps_y2,
                                       scalar=1.0, in1=hb2[:, qt, :],
                                       op0=mybir.AluOpType.mult,
                                       op1=mybir.AluOpType.add,
                                       accum_out=ssum)
        ln_finish(sum2, sum2, ssum)
        nc.sync.dma_start(out=out_re[qt], in_=sum2)

    ctx.close()


def make_in_maps_general(inputs):
    import ml_dtypes
    wd = ml_dtypes.bfloat16

    def pm(a):
        c = a.shape[0] // 128
        return np.ascontiguousarray(
            a.reshape(c, 128, *a.shape[1:]).transpose(1, 0, 2), dtype=wd)

    f = {k: np.asarray(v, dtype=np.float32) for k, v in inputs.items()}
    shared = {
        "w_qm2": np.ascontiguousarray(
            np.stack([pm(f["w_q_m"]), pm(f["w_q_m"])], axis=2), dtype=wd),
        "w_qc2": np.ascontiguousarray(
            np.stack([pm(f["w_q_c"]), pm(f["w_q_c"])], axis=2), dtype=wd),
        "w_k2": np.ascontiguousarray(
            np.stack([pm(f["w_k_m"]), pm(f["w_k_c"])], axis=2), dtype=wd),
        "w_k2s": np.ascontiguousarray(
            np.stack([pm(f["w_k_c"]), pm(f["w_k_m"])], axis=2), dtype=wd),
        "w_v2": np.ascontiguousarray(
            np.stack([pm(f["w_v_m"]), pm(f["w_v_c"])], axis=2), dtype=wd),
        "w_o": np.ascontiguousarray(
            f["w_o"].reshape(H, P, W).transpose(1, 0, 2), dtype=wd),
        "ffn_w1": np.ascontiguousarray(
            f["ffn_w1"].reshape(WC, 128, FC, 128).transpose(1, 2, 0, 3), dtype=wd),
        "ffn_w2": np.ascontiguousarray(
            f["ffn_w2"].reshape(FC, 128, W).transpose(1, 0, 2), dtype=wd),
        "ffn_b1": np.ascontiguousarray(f["ffn_b1"].reshape(FC, 128).T),
        "ln_g": f["ln_g"], "ln_b": f["ln_b"], "ffn_b2": f["ffn_b2"],
    }
    x = f["x"]
    x_cd = x.astype(wd)
    shared["x_t"] = np.ascontiguousarray(x_cd.T)
    in_maps = []
    for c in range(NCORES):
        m = dict(shared)
        xr = x[c * R:(c + 1) * R]
        m["x_rows"] = np.ascontiguousarray(
            xr.reshape(RT, 128, W).transpose(1, 0, 2))
        m["xr_t"] = np.ascontiguousarray(
            x_cd.T[:, c * R:(c + 1) * R].reshape(WC, 128, R).transpose(1, 0, 2))
        in_maps.append(m)
    return in_maps


# ======================================================================
# dispatch
# ======================================================================

_NC_CACHE = {}


def build_nc(mode=MODE):
    return build_nc_fast() if mode == "fast" else build_nc_general()


def get_nc(mode=MODE):
    if mode not in _NC_CACHE:
        nc = build_nc(mode)
        nc.finalize()
        _NC_CACHE[mode] = nc
    return _NC_CACHE[mode]


def make_in_maps(inputs, mode=MODE):
    if mode == "fast":
        return make_in_maps_fast(inputs)
    return make_in_maps_general(inputs)


def pick_mode(inputs):
    if MODE != "fast":
        return MODE
    # fast path assumes the additive LN/FFN biases are zero (true for this
    # problem's setup_inputs); fall back to the general kernel otherwise.
    for k in ("ln_b", "ffn_b1", "ffn_b2"):
        v = np.asarray(inputs[k])
        if np.any(v != 0):
            return "general"
    return "fast"


def kernel(**inputs):
    mode = pick_mode(inputs)
    in_maps = make_in_maps(inputs, mode)
    nc = get_nc(mode)
    res = run_bass_kernel_spmd(nc, in_maps, list(range(NCORES)))
    return np.concatenate([res.results[c]["out"] for c in range(NCORES)], axis=0)
